# revision 2
# baseline (speedup 1.0000x reference)
"""Trainium2 Bass kernel for the MACE 3-body symmetric-contraction block.

Math (identical to the reference einsum chain):
  1. fc1: per-l SO3 linear on irreps_x -> x[n,m,c]  (bias on l=0)
  2. Per (node n, channel c) the 9-vector x enters a symmetric polynomial:
        out[o] = sum_p w1[n,p,c] sum_i   U1[o,i,p] x_i
               + sum_p w2[n,p,c] sum_ai  U2[o,a,i,p] x_a x_i
               + sum_p w3[n,p,c] sum_abi U3[o,a,b,i,p] x_a x_b x_i
     U2/U3 pre-symmetrized over sorted tuples -> monomial basis
        x (9) | m2 = sym x.x (45) | m3 = sym x.x.x (165 = 45 e=0 + 120 e>=1)
  3. D[(o,p), f] = U^T mon; term = G^T (D * w[elem]); final SO3 linear.

v2 dataflow (f = (c, n) c-major, F = 64*128 = 8192 per core):
  - Padded 54-row grid for pairs: d-blocks grouped 3-at-a-time, each group
    padded to its max run length (9/6/3).  Pad rows multiply garbage but
    their U rows are zero.  This makes the xpre/xsuf partition replications
    AFFINE: 3 DMAs each (stride-0 / overlapping-window source APs) straight
    from the DRAM x image -- no DRAM->DRAM staging, no 9-run loads.
  - chunk B (117 rows): m2_pad 0:54 | x 54:63 | m3e0_pad 63:117
    m2_pad   = xpre * xsuf          (DVE, f16 2x)
    m3e0_pad = m2_pad * xsuf        (DVE, f16 2x; triples (a,b,b))
  - chunk A (120 rows): true-packed e>=1 triples, per FT2 f-tile:
    ps1 = sel_m2 @ m2_pad (PE), xrp = sel_x @ x (PE),
    xrepS = Pool copy of xrp, mon_lo = xrepS * ps1 (DVE 1x from psum)
  - element weights: wrep [198, F] f16 loaded early in 2 full-F DMAs;
    D psum -> f16 split ACT (rows 0:128) / Pool (rows 128:198), * wrep on
    DVE at 2x, p-group-summed by the 0/1 G matmul.
  - single-DMA transposes: ysb -> x_d (per l-block, 3 DMAs) and
    termSB -> termT (1 DMA), via raw multi-dim APs.

Sharding: pure node-parallel, 128 nodes/core on 8 cores (SPMD, no
collectives).  Host does marshaling only (gathers/transposes/casts).
"""

import numpy as np

# ---------------------------------------------------------------- constants
NTOT, MD, CD = 1024, 9, 64
P3D, P2D, P1D = 16, 4, 2
NCORES = 8
NLOC = NTOT // NCORES              # 128 nodes per core
FT = 512                           # main-loop f-tile width
FT2 = 1024                         # sel/mon_lo tile width

PAIRS = [(a, a + d) for d in range(MD) for a in range(MD - d)]          # 45
TRIPS = [(a, a + d, a + d + e) for e in range(MD) for d in range(MD - e)
         for a in range(MD - d - e)]                                    # 165
N2, N3 = len(PAIRS), len(TRIPS)
QIDX = {q: k for k, q in enumerate(PAIRS)}
E0 = N2                            # 45 e=0 triples, first in TRIPS
E1 = N3 - E0                       # 120 e>=1 triples
KMON = MD + N2 + N3                # 219 canonical monomial rows
MOUT = MD * (P3D + P2D + P1D)      # 198 rows of D

# padded pair grid: d-groups of 3, run lengths padded to GLEN[g]
GLEN = [9, 6, 3]
GBASE = [0, 27, 45]
KPAD = 54                          # padded pair-grid rows


def padrow(d, a):
    g = d // 3
    return GBASE[g] + (d - 3 * g) * GLEN[g] + a


KB = 64 + KPAD                     # 118 chunk-B rows: m3e0 at aligned base 64
KA = E1                            # 120 chunk-A rows

# chunk-A triples reordered (s=b-a+e, d, a): equal-length a-runs per s make
# the x_i replication affine (one stride-0 DMA per s-block)
TRIPS_A = [(a, a + d, a + s) for s in range(1, MD) for d in range(s)
           for a in range(MD - s)]
ABASE = [0]
for _s in range(1, MD):
    ABASE.append(ABASE[-1] + _s * (MD - _s))   # xrepS s-block row offsets

# packed f16 constant layout (columns of the pk tile)
_PK_ITEMS = (("ua", MOUT), ("ub", MOUT), ("glo", MD), ("ghi", MD),
             ("w1t", 3 * CD), ("wct", 3 * CD), ("sm2", E1))
PK_OFF = {}
_c = 0
for _nm, _w in _PK_ITEMS:
    PK_OFF[_nm] = _c
    _c += _w
PK_BASE = _c                       # xt starts here
W9 = MD * NLOC                     # 1152

_PROGRAM = {}                      # (nloc, repeat, stage) -> compiled program

# schedule/structure knobs (sim-swept)
CFG = dict(work_bufs=3, dwl_mode="mix", dwh_mode="pool", m2_chunks=4,
           wrep_hold=0.014, xrep_eng="hwdge", r1_bufs=2,
           dlo_bufs=2, dhi_bufs=2, t_bufs=2, warmup=0,
           m3e0_pool=True, dwh_mix=True, xps_halves=False,
           m3e0_dve_q=0, m3e0_chunks=4, out_split=True,
           wrep_hold0=0.0)


# ---------------------------------------------------------------- host prep
def _sym_compress(U3, U2):
    tidx = {t: k for k, t in enumerate(TRIPS)}
    U3c = np.zeros((MD, N3, P3D), np.float64)
    for a in range(MD):
        for b in range(MD):
            for i in range(MD):
                U3c[:, tidx[tuple(sorted((a, b, i)))], :] += U3[:, a, b, i, :]
    U2c = np.zeros((MD, N2, P2D), np.float64)
    for a in range(MD):
        for i in range(MD):
            U2c[:, QIDX[tuple(sorted((a, i)))], :] += U2[:, a, i, :]
    return U3c.astype(np.float32), U2c.astype(np.float32)


def _build_u(U3c, U2c, U1):
    # canonical U rows: 0..8 x | 9..53 m2 | 54..218 m3 (TRIPS order)
    # out cols mo:  o*16+p (corr3) | 144+o*4+p (corr2) | 180+o*2+p (corr1)
    tidx = {t: k for k, t in enumerate(TRIPS)}
    U = np.zeros((KMON, MOUT), np.float32)
    G = np.zeros((MOUT, MD), np.float32)
    for o in range(MD):
        U[MD + N2:, o * P3D:(o + 1) * P3D] = U3c[o]
        U[MD:MD + N2, 144 + o * P2D:144 + (o + 1) * P2D] = U2c[o]
        U[0:MD, 180 + o * P1D:180 + (o + 1) * P1D] = U1[o]
        G[o * P3D:(o + 1) * P3D, o] = 1.0
        G[144 + o * P2D:144 + (o + 1) * P2D, o] = 1.0
        G[180 + o * P1D:180 + (o + 1) * P1D, o] = 1.0
    # chunk A: e>=1 m3 rows in TRIPS_A (s, d, a) order
    UA = np.stack([U[MD + N2 + tidx[t]] for t in TRIPS_A])   # [120, 198]
    # chunk B: m2_pad | x | m3e0_pad (pad-garbage rows stay zero)
    UB = np.zeros((KB, MOUT), np.float32)
    for d in range(MD):
        for a in range(MD - d):
            r = padrow(d, a)
            UB[r] = U[MD + QIDX[(a, a + d)]]                          # m2
            UB[64 + r] = U[MD + N2 + tidx[(a, a + d, a + d)]]        # m3e0
    UB[KPAD:KPAD + MD] = U[0:MD]                                      # x
    return UA, UB, G


def _build_sels():
    # e>=1 triples t=(a,b,i) in TRIPS_A order; selection from the padded m2
    # grid (pair (a,b)); the x_i side is a DMA replication, not a matmul
    sm2 = np.zeros((KPAD, E1), np.float32)
    for t, (a, b, i) in enumerate(TRIPS_A):
        sm2[padrow(b - a, a), t] = 1.0
    return sm2


# ---------------------------------------------------------------- device
def _build_program(nloc, repeat=1, stage=6):
    import concourse.bacc as bacc
    from concourse import mybir
    from concourse.ap import AP
    from concourse.tile import TileContext

    f16 = mybir.dt.float16
    f32 = mybir.dt.float32
    AF = mybir.ActivationFunctionType
    F = nloc * CD
    nft = F // FT                  # 16 f-tiles
    w9 = MD * nloc
    lblk = [(0, nloc), (nloc, 4 * nloc), (4 * nloc, 9 * nloc)]
    pkw = PK_BASE + w9

    nc = bacc.Bacc("TRN2", debug=False, enable_asserts=False,
                   num_devices=NCORES, num_swdge_queues=4)

    pk_d = nc.dram_tensor("pk", [128, pkw], f16, kind="ExternalInput").ap()
    b12_d = nc.dram_tensor("b12", [CD, 2], f32, kind="ExternalInput").ap()
    wrep_d = nc.dram_tensor("wrep", [MOUT, F], f16, kind="ExternalInput").ap()
    out_d = nc.dram_tensor("out", [CD, w9], f32, kind="ExternalOutput").ap()
    # x rows in DRAM; rows 9..10 never written (stay zero) so the padded
    # window reads below stay finite
    x_d = nc.dram_tensor("x_sc", [MD + 2, F], f16, kind="Internal").ap()
    termT_d = nc.dram_tensor("termT_sc", [CD, MD * nloc], f16,
                             kind="Internal").ap()

    def rawap(apobj, off_elems, dims):
        return AP(apobj.tensor, apobj.offset + off_elems,
                  [list(d) for d in dims])

    with TileContext(nc) as tc:
        with (
            tc.tile_pool(name="const", bufs=1) as const,
            tc.tile_pool(name="big", bufs=1) as big,
            tc.tile_pool(name="big1", bufs=1) as big1,
            tc.tile_pool(name="work", bufs=CFG["work_bufs"]) as work,
            tc.tile_pool(name="ps_r", bufs=CFG["r1_bufs"], space="PSUM") as ps_r,
            tc.tile_pool(name="ps_d", bufs=CFG["dlo_bufs"], space="PSUM") as ps_d,
            tc.tile_pool(name="ps_h", bufs=CFG["dhi_bufs"], space="PSUM") as ps_h,
            tc.tile_pool(name="ps_t", bufs=CFG["t_bufs"], space="PSUM") as ps_t,
        ):
          def _emit():
            pk = const.tile([128, pkw], f16, name="pk", tag="pk")
            pk32 = const.tile([CD, 2], f32, name="pk32", tag="pk32")

            def pks(nm, r0, r1):
                return pk[r0:r1, PK_OFF[nm]:PK_OFF[nm] + dict(_PK_ITEMS)[nm]]

            ua = pks("ua", 0, KA)
            ub = pks("ub", 0, KB)
            glo = pks("glo", 0, 128)
            ghi = pks("ghi", 0, MOUT - 128)
            w1t = pks("w1t", 0, CD)
            wct = pks("wct", 0, CD)
            sm2 = pks("sm2", 0, KPAD)
            xt = pk[0:CD, PK_BASE:PK_BASE + w9]
            sb_b1 = pk32[:, 0:1]
            sb_b2 = pk32[:, 1:2]

            ysb = big.tile([CD, w9], f16, name="ysb", tag="ysb")
            xpre = big.tile([KPAD, F], f16, name="xpre", tag="xpre")
            xsuf = big.tile([KPAD, F], f16, name="xsuf", tag="xsuf")
            mon_hi = big.tile([KB, F], f16, name="mon_hi", tag="mon_hi")
            mon_lo = big1.tile([KA, F], f16, name="mon_lo", tag="mon_lo")
            xrepS = big1.tile([KA, F], f16, name="xrepS", tag="xrepS")
            wrep_lo = big1.tile([128, F], f16, name="wrep_lo", tag="wrep_lo")
            wrep_hi = big1.tile([MOUT - 128, F], f16, name="wrep_hi",
                                tag="wrep_hi")
            termSB = big1.tile([MD, F], f16, name="termSB", tag="termSB")
            termT = big1.tile([CD, w9], f16, name="termT", tag="termT")
            outSB = big1.tile([CD, w9], f32, name="outSB", tag="outSB")

            # pk split: tiny fc1/sel-critical slice first (only rows 0:64
            # of the w1t..xt columns hold data), U columns second
            pkc0 = PK_OFF["w1t"]
            nc.sync.dma_start(out=pk[0:CD, pkc0:pkw],
                              in_=pk_d[0:CD, pkc0:pkw])
            nc.scalar.dma_start(out=pk[:, 0:pkc0], in_=pk_d[:, 0:pkc0])
            nc.gpsimd.dma_start(out=pk32[:, :], in_=b12_d)
            if stage <= 5:
                nc.gpsimd.memset(outSB[:, :], 0.0)   # early-exit debug only

            # ---------------- fc1 per l + x-row transpose gathers, emitted
            # interleaved so each gather is scheduled as soon as its l-block
            # activation lands
            for l, (c0, c1) in enumerate(lblk):
                w_l = w1t[:, l * CD:(l + 1) * CD]
                for s0 in range(c0, c1, FT):
                    s1 = min(s0 + FT, c1)
                    py = ps_t.tile([CD, FT], f32, name="py", tag="t")
                    nc.tensor.matmul(py[:, :s1 - s0], lhsT=w_l,
                                     rhs=xt[:, s0:s1], start=True, stop=True)
                    if l == 0:
                        nc.scalar.activation(ysb[:, s0:s1], py[:, :s1 - s0],
                                             AF.Identity, bias=sb_b1)
                    else:
                        nc.scalar.activation(ysb[:, s0:s1], py[:, :s1 - s0],
                                             AF.Copy)
                if stage >= 2:
                    m0, w = c0 // nloc, (c1 - c0) // nloc
                    eng = (nc.sync, nc.scalar, nc.sync)[l]
                    eng.dma_start(
                        out=rawap(x_d, m0 * F, [[nloc, CD], [F, w], [1, nloc]]),
                        in_=ysb[:, c0:c1])

            if stage <= 1:
                nc.sync.dma_start(out=out_d, in_=outSB)
                return

            # pad rows 9..10 must hold FINITE values (Internal DRAM is NOT
            # zeroed; NaN garbage would poison psum even under zero U rows)
            nc.gpsimd.dma_start(out=x_d[MD:MD + 2, :],
                                in_=ysb[:, 0:2 * nloc])

            # Replications straight from DRAM (affine APs), column-halved
            # and ordered so half-0 compute overlaps half-1 transfers:
            #   xpre grid g rows (j, a) = x_a        -> stride-0 outer
            #   xsuf grid g rows (j, a) = x_{3g+j+a} -> overlapping window
            #   xrepS s-block rows (d, a) = x_{a+s}  -> stride-0 over d
            nxh = 2 if CFG["xps_halves"] else 1
            wxh = F // nxh
            for ch in range(nxh):
                for g in range(3):
                    L = GLEN[g]
                    b = GBASE[g]
                    eng = (nc.sync, nc.scalar, nc.sync)[g]
                    eng.dma_start(out=xpre[b:b + 3 * L,
                                           ch * wxh:(ch + 1) * wxh],
                                  in_=rawap(x_d, ch * wxh,
                                            [[0, 3], [F, L], [1, wxh]]))
                    eng2 = (nc.scalar, nc.sync, nc.scalar)[g]
                    eng2.dma_start(out=xsuf[b:b + 3 * L,
                                            ch * wxh:(ch + 1) * wxh],
                                   in_=rawap(x_d, 3 * g * F + ch * wxh,
                                             [[F, 3], [F, L], [1, wxh]]))
            nc.gpsimd.dma_start(out=mon_hi[KPAD:64, :],
                                in_=x_d[0:64 - KPAD, :])
            # xrepS in column halves: h0 unblocks tile-0's mon_lo before
            # the wrep chunks take the DMA pool
            h2x = F // 2
            for ch in range(2):
                for s in range(1, MD):
                    if CFG["xrep_eng"] == "pool":
                        eng = nc.gpsimd
                    else:
                        eng = (nc.sync, nc.scalar)[s % 2]
                    with tc.tile_wait_until(0.012 * ch, enable=ch > 0):
                        eng.dma_start(
                            out=xrepS[ABASE[s - 1]:ABASE[s],
                                      ch * h2x:(ch + 1) * h2x],
                            in_=rawap(x_d, s * F + ch * h2x,
                                      [[0, s], [F, MD - s], [1, h2x]]))
            if stage >= 5:
                wh = CFG["wrep_hold"]
                w0 = CFG["wrep_hold0"]
                h2w = F // 2
                for ci in range(2):
                    cw = slice(ci * h2w, (ci + 1) * h2w)
                    h_lo = w0 if ci == 0 else wh + 0.007
                    h_hi = (w0 + 0.002) if ci == 0 else wh + 0.012
                    with tc.tile_wait_until(h_lo, enable=h_lo > 0):
                        nc.sync.dma_start(out=wrep_lo[:, cw],
                                          in_=wrep_d[0:128, cw])
                    with tc.tile_wait_until(h_hi, enable=h_hi > 0):
                        nc.scalar.dma_start(out=wrep_hi[:, cw],
                                            in_=wrep_d[128:MOUT, cw])

            if stage <= 2:
                nc.sync.dma_start(out=out_d, in_=outSB)
                return

            # PE warm-up: keep the tensor engine busy through the DMA head
            # so the main loop runs at full p-state from its first matmul
            for wi in range(CFG["warmup"]):
                with tc.tile_wait_until(0.006 + 0.002 * wi):
                    pw = ps_t.tile([CD, FT], f32, name="pw", tag="t")
                    nc.tensor.matmul(pw[:, :], lhsT=w1t[:, 0:CD],
                                     rhs=xt[:, 0:FT], start=True, stop=True)

            # m2_pad and m3e0_pad, column-chunked for earlier availability;
            # m3e0 optionally on Pool (slow but otherwise idle in the head)
            nq = CFG["m2_chunks"]
            q4 = F // nq
            for qi in range(nq):
                cs = slice(qi * q4, (qi + 1) * q4)
                nc.vector.tensor_mul(mon_hi[0:KPAD, cs], xpre[:, cs],
                                     xsuf[:, cs])
            nq3 = CFG["m3e0_chunks"]
            q3 = F // nq3
            for qi in range(nq3):
                cs = slice(qi * q3, (qi + 1) * q3)
                if CFG["m3e0_pool"] and qi >= CFG["m3e0_dve_q"]:
                    nc.gpsimd.tensor_mul(mon_hi[64:KB, cs],
                                         mon_hi[0:KPAD, cs], xsuf[:, cs])
                else:
                    nc.vector.tensor_mul(mon_hi[64:KB, cs],
                                         mon_hi[0:KPAD, cs], xsuf[:, cs])

            if stage <= 3:
                nc.sync.dma_start(out=out_d, in_=outSB)
                return

            # ---------------- main loop at FT: sel+mon_lo inline
            for p in range(nft):
                    js = slice(p * FT, (p + 1) * FT)
                    ps1 = ps_r.tile([KA, FT], f32, name="ps1", tag="r1")
                    nc.tensor.matmul(ps1[:, :], lhsT=sm2,
                                     rhs=mon_hi[0:KPAD, js],
                                     start=True, stop=True)
                    nc.vector.tensor_mul(mon_lo[:, js], xrepS[:, js],
                                         ps1[:, :])
                    if stage <= 4:
                        continue
                    dlo = ps_d.tile([128, FT], f32, name="dlo", tag="dlo")
                    dhi = ps_h.tile([MOUT - 128, FT], f32, name="dhi",
                                    tag="dhi")
                    nc.tensor.matmul(dlo[:, :], lhsT=ua[:, 0:128],
                                     rhs=mon_lo[:, js], start=True,
                                     stop=False)
                    nc.tensor.matmul(dlo[:, :], lhsT=ub[:, 0:128],
                                     rhs=mon_hi[0:KB, js],
                                     start=False, stop=True)
                    nc.tensor.matmul(dhi[:, :], lhsT=ua[:, 128:MOUT],
                                     rhs=mon_lo[:, js], start=True,
                                     stop=False)
                    nc.tensor.matmul(dhi[:, :], lhsT=ub[:, 128:MOUT],
                                     rhs=mon_hi[0:KB, js],
                                     start=False, stop=True)
                    dwl = work.tile([128, FT], f16, name="dwl", tag="dwl")
                    dwh = work.tile([MOUT - 128, FT], f16, name="dwh",
                                    tag="dwh")
                    dm = CFG["dwl_mode"]
                    if dm == "direct" or (dm == "mix" and p % 2 == 1):
                        nc.vector.tensor_mul(dwl[:, :], dlo[:, :],
                                             wrep_lo[:, js])
                    else:
                        dsl = work.tile([128, FT], f16, name="dsl",
                                        tag="dsl")
                        nc.scalar.activation(dsl[:, :], dlo[:, :], AF.Copy)
                        nc.vector.tensor_mul(dwl[:, :], dsl[:, :],
                                             wrep_lo[:, js])
                    if CFG["dwh_mode"] == "pool" and not (
                            CFG["dwh_mix"] and p % 2 == 1):
                        dsh = work.tile([MOUT - 128, FT], f16, name="dsh",
                                        tag="dsh")
                        nc.scalar.activation(dsh[:, :], dhi[:, :], AF.Copy)
                        nc.gpsimd.tensor_mul(dwh[:, :], dsh[:, :],
                                             wrep_hi[:, js])
                    else:
                        nc.vector.tensor_mul(dwh[:, :], dhi[:, :],
                                             wrep_hi[:, js])
                    pt = ps_t.tile([MD, FT], f32, name="pt", tag="t")
                    nc.tensor.matmul(pt[:, :], lhsT=glo, rhs=dwl[:, :],
                                     start=True, stop=False)
                    nc.tensor.matmul(pt[:, :], lhsT=ghi, rhs=dwh[:, :],
                                     start=False, stop=True)
                    nc.scalar.activation(termSB[:, js], pt[:, :], AF.Copy)

            if stage <= 5:
                nc.sync.dma_start(out=out_d, in_=outSB)
                return

            # tail: per-o transpose gathers, pipelined with per-l finals
            for l, (c0, c1) in enumerate(lblk):
                for o in range(c0 // nloc, c1 // nloc):
                    dst = termT[:, o * nloc:(o + 1) * nloc]
                    eng = (nc.sync, nc.scalar, nc.gpsimd)[o % 3]
                    eng.dma_start(out=dst, in_=termSB[o:o + 1, :])
                w_l = wct[:, l * CD:(l + 1) * CD]
                for s0 in range(c0, c1, FT):
                    s1 = min(s0 + FT, c1)
                    pf = ps_t.tile([CD, FT], f32, name="pf", tag="t")
                    nc.tensor.matmul(pf[:, :s1 - s0], lhsT=w_l,
                                     rhs=termT[:, s0:s1], start=True,
                                     stop=True)
                    if l == 0:
                        nc.scalar.activation(outSB[:, s0:s1], pf[:, :s1 - s0],
                                             AF.Identity, bias=sb_b2)
                    else:
                        nc.scalar.activation(outSB[:, s0:s1], pf[:, :s1 - s0],
                                             AF.Copy)
                if CFG["out_split"]:
                    eng = (nc.sync, nc.scalar, nc.sync)[l]
                    eng.dma_start(out=out_d[:, c0:c1], in_=outSB[:, c0:c1])
            if not CFG["out_split"]:
                nc.sync.dma_start(out=out_d, in_=outSB[:, :])

          if repeat > 1:
              with tc.For_i(0, repeat, 1):
                  _emit()
          else:
              _emit()

    return nc


def _get_program(nloc, repeat=1, stage=6):
    key = (nloc, repeat, stage)
    if key not in _PROGRAM:
        nc = _build_program(nloc, repeat, stage)
        nc.compile()
        _PROGRAM[key] = nc
    return _PROGRAM[key]


def make_in_maps(irreps_x, atomic_numbers, w_fc1, b_fc1, U3, W3, U2, W2, U1,
                 W1, w_lin, w_fc2, b_fc2, nloc=NLOC, ncores=NCORES):
    irreps_x = np.asarray(irreps_x, np.float32)
    a_n = np.asarray(atomic_numbers).astype(np.int64)
    U3c, U2c = _sym_compress(np.asarray(U3, np.float64),
                             np.asarray(U2, np.float64))
    UA, UB, G = _build_u(U3c, U2c, np.asarray(U1, np.float32))
    sm2 = _build_sels()
    w_comb = np.einsum('lde,lec->ldc', np.asarray(w_fc2, np.float32),
                       np.asarray(w_lin, np.float32))
    w1t = np.concatenate([np.asarray(w_fc1, np.float32)[l].T
                          for l in range(3)], axis=1)
    wct = np.concatenate([w_comb[l].T for l in range(3)], axis=1)
    w3g = np.asarray(W3, np.float32)[a_n]          # [N, 16, 64]
    w2g = np.asarray(W2, np.float32)[a_n]
    w1g = np.asarray(W1, np.float32)[a_n]
    F = nloc * CD

    def put(buf, nm, arr, r0=0):
        o = PK_OFF[nm]
        arr = np.asarray(arr, np.float32).astype(np.float16)
        buf[r0:r0 + arr.shape[0], o:o + arr.shape[1]] = arr

    b12 = np.stack([np.asarray(b_fc1, np.float32),
                    np.asarray(b_fc2, np.float32)], axis=1).astype(np.float32)
    in_maps = []
    for core in range(ncores):
        s = slice(core * nloc, (core + 1) * nloc)
        parts = []
        for l in range(3):
            seg = irreps_x[s, l * l:(l + 1) * (l + 1), :]   # [nloc, w, 64]
            parts.append(seg.transpose(2, 1, 0).reshape(CD, -1))
        xtc = np.concatenate(parts, axis=1)                 # [64, 9*nloc]
        pk = np.zeros((128, PK_BASE + MD * nloc), np.float16)
        put(pk, "ua", UA)
        put(pk, "ub", UB)
        put(pk, "glo", G[0:128])
        put(pk, "ghi", G[128:MOUT])
        put(pk, "w1t", w1t)
        put(pk, "wct", wct)
        put(pk, "sm2", sm2)
        pk[:CD, PK_BASE:PK_BASE + MD * nloc] = xtc.astype(np.float16)
        wg3 = w3g[s].transpose(1, 2, 0).reshape(P3D, F)     # [16, F] f=(c,n)
        wg2 = w2g[s].transpose(1, 2, 0).reshape(P2D, F)
        wg1 = w1g[s].transpose(1, 2, 0).reshape(P1D, F)
        wrep = np.concatenate([np.tile(wg3, (MD, 1)), np.tile(wg2, (MD, 1)),
                               np.tile(wg1, (MD, 1))], axis=0)  # [198, F]
        in_maps.append({
            "pk": pk,
            "b12": b12,
            "wrep": wrep.astype(np.float16),
        })
    return in_maps


def unpack_out(o, nloc=NLOC):
    # o: [64, 9*nloc] cols (o, n) o-major -> [nloc, 9, 64]
    return np.ascontiguousarray(
        o.reshape(CD, MD, nloc).transpose(2, 1, 0)).astype(np.float32)


# ---------------------------------------------------------------- entry
def kernel(**inputs):
    from concourse import bass_utils
    in_maps = make_in_maps(**inputs)
    nc = _get_program(NLOC)
    res = bass_utils.run_bass_kernel_spmd(nc, in_maps,
                                          core_ids=list(range(NCORES)))
    outs = [unpack_out(res.results[c]["out"]) for c in range(NCORES)]
    return np.concatenate(outs, axis=0).astype(np.float32)


# revision 5
# speedup vs baseline: 1.1451x; 1.1451x over previous
"""Trainium2 Bass kernel for the MACE 3-body symmetric-contraction block.

Math (identical to the reference einsum chain):
  1. fc1: per-l SO3 linear on irreps_x -> x[n,m,c]  (bias on l=0)
  2. Per (node n, channel c) the 9-vector x enters a symmetric polynomial:
        out[o] = sum_p w1[n,p,c] sum_i   U1[o,i,p] x_i
               + sum_p w2[n,p,c] sum_ai  U2[o,a,i,p] x_a x_i
               + sum_p w3[n,p,c] sum_abi U3[o,a,b,i,p] x_a x_b x_i
     U2/U3 pre-symmetrized over sorted tuples -> monomial basis
        x (9) | m2 = sym x.x (45) | m3 = sym x.x.x (165 = 45 e=0 + 120 e>=1)
  3. D[(o,p), f] = U^T mon; term = G^T (D * w[elem]); final SO3 linear.

v2 dataflow (f = (c, n) c-major, F = 64*128 = 8192 per core):
  - Padded 54-row grid for pairs: d-blocks grouped 3-at-a-time, each group
    padded to its max run length (9/6/3).  Pad rows multiply garbage but
    their U rows are zero.  This makes the xpre/xsuf partition replications
    AFFINE: 3 DMAs each (stride-0 / overlapping-window source APs) straight
    from the DRAM x image -- no DRAM->DRAM staging, no 9-run loads.
  - chunk B (117 rows): m2_pad 0:54 | x 54:63 | m3e0_pad 63:117
    m2_pad   = xpre * xsuf          (DVE, f16 2x)
    m3e0_pad = m2_pad * xsuf        (DVE, f16 2x; triples (a,b,b))
  - chunk A (120 rows): e>=1 triples reordered (s=b-a+e, d, a) so the x_i
    replication is AFFINE (one stride-0 DMA per s-block, column-halved);
    per FT tile: ps1 = sel_m2 @ m2_pad (PE), mon_lo = xrepS * ps1 (DVE).
    m3e0 runs on Pool (slow but otherwise idle through the head).
  - element weights: wrep [198, F] f16 in 4 held column-chunk DMAs so the
    region-tracked dw consumers unblock per chunk; D psum -> f16 via ACT
    (rows 0:128) then * wrep on DVE at 2x; rows 128:198 multiply straight
    from psum on DVE (early tiles) or via ACT copy + Pool mul (late);
    p-group-summed by the 0/1 G matmul.
  - transposes via multi-dim DRAM-side APs: ysb -> x_d in 3 DMAs (one per
    l-block); termSB -> termT as 9 per-o gathers pipelined with the final
    per-l linear (l=2 first).
  - schedule shaped with tile_wait_until holds so the DMA pool serves the
    critical replication chain before the bulk wrep stream.

Sharding: pure node-parallel, 128 nodes/core on 8 cores (SPMD, no
collectives).  Host does marshaling only (gathers/transposes/casts).
"""

import numpy as np

# ---------------------------------------------------------------- constants
NTOT, MD, CD = 1024, 9, 64
P3D, P2D, P1D = 16, 4, 2
NCORES = 8
NLOC = NTOT // NCORES              # 128 nodes per core
FT = 512                           # main-loop f-tile width
FT2 = 1024                         # sel/mon_lo tile width

PAIRS = [(a, a + d) for d in range(MD) for a in range(MD - d)]          # 45
TRIPS = [(a, a + d, a + d + e) for e in range(MD) for d in range(MD - e)
         for a in range(MD - d - e)]                                    # 165
N2, N3 = len(PAIRS), len(TRIPS)
QIDX = {q: k for k, q in enumerate(PAIRS)}
E0 = N2                            # 45 e=0 triples, first in TRIPS
E1 = N3 - E0                       # 120 e>=1 triples
KMON = MD + N2 + N3                # 219 canonical monomial rows
MOUT = MD * (P3D + P2D + P1D)      # 198 rows of D

# padded pair grid: d-groups of 3, run lengths padded to GLEN[g]
GLEN = [9, 6, 3]
GBASE = [0, 27, 45]
KPAD = 54                          # padded pair-grid rows


def padrow(d, a):
    g = d // 3
    return GBASE[g] + (d - 3 * g) * GLEN[g] + a


KB = 64 + KPAD                     # 118 chunk-B rows: m3e0 at aligned base 64
KA = E1                            # 120 chunk-A rows

# chunk-A triples reordered (s=b-a+e, d, a): equal-length a-runs per s make
# the x_i replication affine (one stride-0 DMA per s-block)
TRIPS_A = [(a, a + d, a + s) for s in range(1, MD) for d in range(s)
           for a in range(MD - s)]
ABASE = [0]
for _s in range(1, MD):
    ABASE.append(ABASE[-1] + _s * (MD - _s))   # xrepS s-block row offsets

# packed f16 constant layout (columns of the pk tile)
_PK_ITEMS = (("ua", MOUT), ("ub", MOUT), ("glo", MD), ("ghi", MD),
             ("w1t", 3 * CD), ("wct", 3 * CD), ("sm2", E1))
PK_OFF = {}
_c = 0
for _nm, _w in _PK_ITEMS:
    PK_OFF[_nm] = _c
    _c += _w
PK_BASE = _c                       # xt starts here
W9 = MD * NLOC                     # 1152

_PROGRAM = {}                      # (nloc, repeat, stage) -> compiled program

# schedule/structure knobs (sim-swept)
CFG = dict(work_bufs=3, dwl_mode="mix", dwh_mode="pool", m2_chunks=4,
           wrep_hold=0.014, xrep_eng="hwdge", r1_bufs=2,
           dlo_bufs=2, dhi_bufs=2, t_bufs=2, warmup=0,
           m3e0_pool=True, dwh_mix=True, xps_halves=False,
           m3e0_dve_q=0, m3e0_chunks=4, out_split=True,
           wrep_hold0=0.014, warm_t0=0.013, dwh_pool_from=10,
           monlo_2x=False)


# ---------------------------------------------------------------- host prep
def _sym_compress(U3, U2):
    tidx = {t: k for k, t in enumerate(TRIPS)}
    U3c = np.zeros((MD, N3, P3D), np.float64)
    for a in range(MD):
        for b in range(MD):
            for i in range(MD):
                U3c[:, tidx[tuple(sorted((a, b, i)))], :] += U3[:, a, b, i, :]
    U2c = np.zeros((MD, N2, P2D), np.float64)
    for a in range(MD):
        for i in range(MD):
            U2c[:, QIDX[tuple(sorted((a, i)))], :] += U2[:, a, i, :]
    return U3c.astype(np.float32), U2c.astype(np.float32)


def _build_u(U3c, U2c, U1):
    # canonical U rows: 0..8 x | 9..53 m2 | 54..218 m3 (TRIPS order)
    # out cols mo:  o*16+p (corr3) | 144+o*4+p (corr2) | 180+o*2+p (corr1)
    tidx = {t: k for k, t in enumerate(TRIPS)}
    U = np.zeros((KMON, MOUT), np.float32)
    G = np.zeros((MOUT, MD), np.float32)
    for o in range(MD):
        U[MD + N2:, o * P3D:(o + 1) * P3D] = U3c[o]
        U[MD:MD + N2, 144 + o * P2D:144 + (o + 1) * P2D] = U2c[o]
        U[0:MD, 180 + o * P1D:180 + (o + 1) * P1D] = U1[o]
        G[o * P3D:(o + 1) * P3D, o] = 1.0
        G[144 + o * P2D:144 + (o + 1) * P2D, o] = 1.0
        G[180 + o * P1D:180 + (o + 1) * P1D, o] = 1.0
    # chunk A: e>=1 m3 rows in TRIPS_A (s, d, a) order
    UA = np.stack([U[MD + N2 + tidx[t]] for t in TRIPS_A])   # [120, 198]
    # chunk B: m2_pad | x | m3e0_pad (pad-garbage rows stay zero)
    UB = np.zeros((KB, MOUT), np.float32)
    for d in range(MD):
        for a in range(MD - d):
            r = padrow(d, a)
            UB[r] = U[MD + QIDX[(a, a + d)]]                          # m2
            UB[64 + r] = U[MD + N2 + tidx[(a, a + d, a + d)]]        # m3e0
    UB[KPAD:KPAD + MD] = U[0:MD]                                      # x
    return UA, UB, G


def _build_sels():
    # e>=1 triples t=(a,b,i) in TRIPS_A order; selection from the padded m2
    # grid (pair (a,b)); the x_i side is a DMA replication, not a matmul
    sm2 = np.zeros((KPAD, E1), np.float32)
    for t, (a, b, i) in enumerate(TRIPS_A):
        sm2[padrow(b - a, a), t] = 1.0
    return sm2


# ---------------------------------------------------------------- device
def _build_program(nloc, repeat=1, stage=6):
    import concourse.bacc as bacc
    from concourse import mybir
    from concourse.ap import AP
    from concourse.tile import TileContext

    f16 = mybir.dt.float16
    f32 = mybir.dt.float32
    AF = mybir.ActivationFunctionType
    F = nloc * CD
    nft = F // FT                  # 16 f-tiles
    w9 = MD * nloc
    lblk = [(0, nloc), (nloc, 4 * nloc), (4 * nloc, 9 * nloc)]
    pkw = PK_BASE + w9

    nc = bacc.Bacc("TRN2", debug=False, enable_asserts=False,
                   num_devices=NCORES, num_swdge_queues=4)

    pk_d = nc.dram_tensor("pk", [128, pkw], f16, kind="ExternalInput").ap()
    b12_d = nc.dram_tensor("b12", [CD, 2], f32, kind="ExternalInput").ap()
    wrep_d = nc.dram_tensor("wrep", [MOUT, F], f16, kind="ExternalInput").ap()
    out_d = nc.dram_tensor("out", [CD, w9], f32, kind="ExternalOutput").ap()
    # x rows in DRAM; rows 9..10 never written (stay zero) so the padded
    # window reads below stay finite
    x_d = nc.dram_tensor("x_sc", [MD + 2, F], f16, kind="Internal").ap()
    termT_d = nc.dram_tensor("termT_sc", [CD, MD * nloc], f16,
                             kind="Internal").ap()

    def rawap(apobj, off_elems, dims):
        return AP(apobj.tensor, apobj.offset + off_elems,
                  [list(d) for d in dims])

    with TileContext(nc) as tc:
        with (
            tc.tile_pool(name="const", bufs=1) as const,
            tc.tile_pool(name="big", bufs=1) as big,
            tc.tile_pool(name="big1", bufs=1) as big1,
            tc.tile_pool(name="work", bufs=CFG["work_bufs"]) as work,
            tc.tile_pool(name="ps_r", bufs=CFG["r1_bufs"], space="PSUM") as ps_r,
            tc.tile_pool(name="ps_d", bufs=CFG["dlo_bufs"], space="PSUM") as ps_d,
            tc.tile_pool(name="ps_h", bufs=CFG["dhi_bufs"], space="PSUM") as ps_h,
            tc.tile_pool(name="ps_t", bufs=CFG["t_bufs"], space="PSUM") as ps_t,
        ):
          def _emit():
            pk = const.tile([128, pkw], f16, name="pk", tag="pk")
            pk32 = const.tile([CD, 2], f32, name="pk32", tag="pk32")

            def pks(nm, r0, r1):
                return pk[r0:r1, PK_OFF[nm]:PK_OFF[nm] + dict(_PK_ITEMS)[nm]]

            ua = pks("ua", 0, KA)
            ub = pks("ub", 0, KB)
            glo = pks("glo", 0, 128)
            ghi = pks("ghi", 0, MOUT - 128)
            w1t = pks("w1t", 0, CD)
            wct = pks("wct", 0, CD)
            sm2 = pks("sm2", 0, KPAD)
            xt = pk[0:CD, PK_BASE:PK_BASE + w9]
            sb_b1 = pk32[:, 0:1]
            sb_b2 = pk32[:, 1:2]

            ysb = big.tile([CD, w9], f16, name="ysb", tag="ysb")
            xpre = big.tile([KPAD, F], f16, name="xpre", tag="xpre")
            xsuf = big.tile([KPAD, F], f16, name="xsuf", tag="xsuf")
            mon_hi = big.tile([KB, F], f16, name="mon_hi", tag="mon_hi")
            mon_lo = big1.tile([KA, F], f16, name="mon_lo", tag="mon_lo")
            xrepS = big1.tile([KA, F], f16, name="xrepS", tag="xrepS")
            wrep_lo = big1.tile([128, F], f16, name="wrep_lo", tag="wrep_lo")
            wrep_hi = big1.tile([MOUT - 128, F], f16, name="wrep_hi",
                                tag="wrep_hi")
            termSB = big1.tile([MD, F], f16, name="termSB", tag="termSB")
            termT = big1.tile([CD, w9], f16, name="termT", tag="termT")
            outSB = big1.tile([CD, w9], f32, name="outSB", tag="outSB")

            # pk split: tiny fc1/sel-critical slice first (only rows 0:64
            # of the w1t..xt columns hold data), U columns second
            pkc0 = PK_OFF["w1t"]
            nc.sync.dma_start(out=pk[0:CD, pkc0:pkw],
                              in_=pk_d[0:CD, pkc0:pkw])
            nc.scalar.dma_start(out=pk[:, 0:pkc0], in_=pk_d[:, 0:pkc0])
            nc.gpsimd.dma_start(out=pk32[:, :], in_=b12_d)
            if stage <= 5:
                nc.gpsimd.memset(outSB[:, :], 0.0)   # early-exit debug only

            # ---------------- fc1 per l + x-row transpose gathers, emitted
            # interleaved so each gather is scheduled as soon as its l-block
            # activation lands
            for l, (c0, c1) in enumerate(lblk):
                w_l = w1t[:, l * CD:(l + 1) * CD]
                for s0 in range(c0, c1, FT):
                    s1 = min(s0 + FT, c1)
                    py = ps_t.tile([CD, FT], f32, name="py", tag="t")
                    nc.tensor.matmul(py[:, :s1 - s0], lhsT=w_l,
                                     rhs=xt[:, s0:s1], start=True, stop=True)
                    if l == 0:
                        nc.scalar.activation(ysb[:, s0:s1], py[:, :s1 - s0],
                                             AF.Identity, bias=sb_b1)
                    else:
                        nc.scalar.activation(ysb[:, s0:s1], py[:, :s1 - s0],
                                             AF.Copy)
                if stage >= 2:
                    m0, w = c0 // nloc, (c1 - c0) // nloc
                    eng = (nc.sync, nc.scalar, nc.sync)[l]
                    eng.dma_start(
                        out=rawap(x_d, m0 * F, [[nloc, CD], [F, w], [1, nloc]]),
                        in_=ysb[:, c0:c1])

            if stage <= 1:
                nc.sync.dma_start(out=out_d, in_=outSB)
                return

            # pad rows 9..10 must hold FINITE values (Internal DRAM is NOT
            # zeroed; NaN garbage would poison psum even under zero U rows)
            nc.gpsimd.dma_start(out=x_d[MD:MD + 2, :],
                                in_=ysb[:, 0:2 * nloc])

            # Replications straight from DRAM (affine APs), column-halved
            # and ordered so half-0 compute overlaps half-1 transfers:
            #   xpre grid g rows (j, a) = x_a        -> stride-0 outer
            #   xsuf grid g rows (j, a) = x_{3g+j+a} -> overlapping window
            #   xrepS s-block rows (d, a) = x_{a+s}  -> stride-0 over d
            nxh = 2 if CFG["xps_halves"] else 1
            wxh = F // nxh
            for ch in range(nxh):
                for g in range(3):
                    L = GLEN[g]
                    b = GBASE[g]
                    eng = (nc.sync, nc.scalar, nc.sync)[g]
                    eng.dma_start(out=xpre[b:b + 3 * L,
                                           ch * wxh:(ch + 1) * wxh],
                                  in_=rawap(x_d, ch * wxh,
                                            [[0, 3], [F, L], [1, wxh]]))
                    eng2 = (nc.scalar, nc.sync, nc.scalar)[g]
                    eng2.dma_start(out=xsuf[b:b + 3 * L,
                                            ch * wxh:(ch + 1) * wxh],
                                   in_=rawap(x_d, 3 * g * F + ch * wxh,
                                             [[F, 3], [F, L], [1, wxh]]))
            nc.gpsimd.dma_start(out=mon_hi[KPAD:64, :],
                                in_=x_d[0:64 - KPAD, :])
            # xrepS in column halves: h0 unblocks tile-0's mon_lo before
            # the wrep chunks take the DMA pool
            h2x = F // 2
            for ch in range(2):
                for s in range(1, MD):
                    if CFG["xrep_eng"] == "pool":
                        eng = nc.gpsimd
                    else:
                        eng = (nc.sync, nc.scalar)[s % 2]
                    with tc.tile_wait_until(0.012 * ch, enable=ch > 0):
                        eng.dma_start(
                            out=xrepS[ABASE[s - 1]:ABASE[s],
                                      ch * h2x:(ch + 1) * h2x],
                            in_=rawap(x_d, s * F + ch * h2x,
                                      [[0, s], [F, MD - s], [1, h2x]]))
            if stage >= 5:
                wh = CFG["wrep_hold"]
                w0 = CFG["wrep_hold0"]
                h2w = F // 2
                for ci in range(2):
                    cw = slice(ci * h2w, (ci + 1) * h2w)
                    h_lo = w0 if ci == 0 else wh + 0.007
                    h_hi = (w0 + 0.002) if ci == 0 else wh + 0.012
                    with tc.tile_wait_until(h_lo, enable=h_lo > 0):
                        nc.sync.dma_start(out=wrep_lo[:, cw],
                                          in_=wrep_d[0:128, cw])
                    with tc.tile_wait_until(h_hi, enable=h_hi > 0):
                        nc.scalar.dma_start(out=wrep_hi[:, cw],
                                            in_=wrep_d[128:MOUT, cw])

            if stage <= 2:
                nc.sync.dma_start(out=out_d, in_=outSB)
                return

            # PE warm-up: dense burst just before the main loop so the
            # tensor engine enters it at full p-state (ramp needs ~3us of
            # continuous busy)
            for wi in range(CFG["warmup"]):
                with tc.tile_wait_until(CFG["warm_t0"] + 0.0005 * wi):
                    pw = ps_t.tile([CD, FT], f32, name="pw", tag="t")
                    nc.tensor.matmul(pw[:, :], lhsT=w1t[:, 0:CD],
                                     rhs=xt[:, 0:FT], start=True, stop=True)

            # m2_pad and m3e0_pad, column-chunked for earlier availability;
            # m3e0 optionally on Pool (slow but otherwise idle in the head)
            nq = CFG["m2_chunks"]
            q4 = F // nq
            for qi in range(nq):
                cs = slice(qi * q4, (qi + 1) * q4)
                nc.vector.tensor_mul(mon_hi[0:KPAD, cs], xpre[:, cs],
                                     xsuf[:, cs])
            nq3 = CFG["m3e0_chunks"]
            q3 = F // nq3
            for qi in range(nq3):
                cs = slice(qi * q3, (qi + 1) * q3)
                if CFG["m3e0_pool"] and qi >= CFG["m3e0_dve_q"]:
                    nc.gpsimd.tensor_mul(mon_hi[64:KB, cs],
                                         mon_hi[0:KPAD, cs], xsuf[:, cs])
                else:
                    nc.vector.tensor_mul(mon_hi[64:KB, cs],
                                         mon_hi[0:KPAD, cs], xsuf[:, cs])

            if stage <= 3:
                nc.sync.dma_start(out=out_d, in_=outSB)
                return

            # ---------------- main loop at FT: sel+mon_lo inline
            for p in range(nft):
                    js = slice(p * FT, (p + 1) * FT)
                    ps1 = ps_r.tile([KA, FT], f32, name="ps1", tag="r1")
                    nc.tensor.matmul(ps1[:, :], lhsT=sm2,
                                     rhs=mon_hi[0:KPAD, js],
                                     start=True, stop=True)
                    if CFG["monlo_2x"]:
                        ps1f = work.tile([KA, FT], f16, name="ps1f",
                                         tag="ps1f")
                        nc.scalar.activation(ps1f[:, :], ps1[:, :], AF.Copy)
                        nc.vector.tensor_mul(mon_lo[:, js], xrepS[:, js],
                                             ps1f[:, :])
                    else:
                        nc.vector.tensor_mul(mon_lo[:, js], xrepS[:, js],
                                             ps1[:, :])
                    if stage <= 4:
                        continue
                    dlo = ps_d.tile([128, FT], f32, name="dlo", tag="dlo")
                    dhi = ps_h.tile([MOUT - 128, FT], f32, name="dhi",
                                    tag="dhi")
                    nc.tensor.matmul(dlo[:, :], lhsT=ua[:, 0:128],
                                     rhs=mon_lo[:, js], start=True,
                                     stop=False)
                    nc.tensor.matmul(dlo[:, :], lhsT=ub[:, 0:128],
                                     rhs=mon_hi[0:KB, js],
                                     start=False, stop=True)
                    nc.tensor.matmul(dhi[:, :], lhsT=ua[:, 128:MOUT],
                                     rhs=mon_lo[:, js], start=True,
                                     stop=False)
                    nc.tensor.matmul(dhi[:, :], lhsT=ub[:, 128:MOUT],
                                     rhs=mon_hi[0:KB, js],
                                     start=False, stop=True)
                    dwl = work.tile([128, FT], f16, name="dwl", tag="dwl")
                    dwh = work.tile([MOUT - 128, FT], f16, name="dwh",
                                    tag="dwh")
                    dm = CFG["dwl_mode"]
                    if dm == "direct" or (dm == "mix" and p % 2 == 1):
                        nc.vector.tensor_mul(dwl[:, :], dlo[:, :],
                                             wrep_lo[:, js])
                    else:
                        dsl = work.tile([128, FT], f16, name="dsl",
                                        tag="dsl")
                        nc.scalar.activation(dsl[:, :], dlo[:, :], AF.Copy)
                        nc.vector.tensor_mul(dwl[:, :], dsl[:, :],
                                             wrep_lo[:, js])
                    if CFG["dwh_mode"] == "pool" and not (
                            (CFG["dwh_mix"] and p % 2 == 1) or
                            p < CFG["dwh_pool_from"]):
                        dsh = work.tile([MOUT - 128, FT], f16, name="dsh",
                                        tag="dsh")
                        nc.scalar.activation(dsh[:, :], dhi[:, :], AF.Copy)
                        nc.gpsimd.tensor_mul(dwh[:, :], dsh[:, :],
                                             wrep_hi[:, js])
                    else:
                        nc.vector.tensor_mul(dwh[:, :], dhi[:, :],
                                             wrep_hi[:, js])
                    pt = ps_t.tile([MD, FT], f32, name="pt", tag="t")
                    nc.tensor.matmul(pt[:, :], lhsT=glo, rhs=dwl[:, :],
                                     start=True, stop=False)
                    nc.tensor.matmul(pt[:, :], lhsT=ghi, rhs=dwh[:, :],
                                     start=False, stop=True)
                    nc.scalar.activation(termSB[:, js], pt[:, :], AF.Copy)

            if stage <= 5:
                nc.sync.dma_start(out=out_d, in_=outSB)
                return

            # tail: per-o transpose gathers, pipelined with per-l finals;
            # l=2 first so the largest output chunk stores earliest
            for l, (c0, c1) in sorted(enumerate(lblk), key=lambda x: -x[0]):
                for o in range(c0 // nloc, c1 // nloc):
                    dst = termT[:, o * nloc:(o + 1) * nloc]
                    eng = (nc.sync, nc.scalar, nc.gpsimd)[o % 3]
                    eng.dma_start(out=dst, in_=termSB[o:o + 1, :])
                w_l = wct[:, l * CD:(l + 1) * CD]
                for s0 in range(c0, c1, FT):
                    s1 = min(s0 + FT, c1)
                    pf = ps_t.tile([CD, FT], f32, name="pf", tag="t")
                    nc.tensor.matmul(pf[:, :s1 - s0], lhsT=w_l,
                                     rhs=termT[:, s0:s1], start=True,
                                     stop=True)
                    if l == 0:
                        nc.scalar.activation(outSB[:, s0:s1], pf[:, :s1 - s0],
                                             AF.Identity, bias=sb_b2)
                    else:
                        nc.scalar.activation(outSB[:, s0:s1], pf[:, :s1 - s0],
                                             AF.Copy)
                if CFG["out_split"]:
                    eng = (nc.sync, nc.scalar, nc.sync)[l]
                    eng.dma_start(out=out_d[:, c0:c1], in_=outSB[:, c0:c1])
            if not CFG["out_split"]:
                nc.sync.dma_start(out=out_d, in_=outSB[:, :])

          if repeat > 1:
              with tc.For_i(0, repeat, 1):
                  _emit()
          else:
              _emit()

    return nc


def _get_program(nloc, repeat=1, stage=6):
    key = (nloc, repeat, stage)
    if key not in _PROGRAM:
        nc = _build_program(nloc, repeat, stage)
        nc.compile()
        _PROGRAM[key] = nc
    return _PROGRAM[key]


def make_in_maps(irreps_x, atomic_numbers, w_fc1, b_fc1, U3, W3, U2, W2, U1,
                 W1, w_lin, w_fc2, b_fc2, nloc=NLOC, ncores=NCORES):
    irreps_x = np.asarray(irreps_x, np.float32)
    a_n = np.asarray(atomic_numbers).astype(np.int64)
    U3c, U2c = _sym_compress(np.asarray(U3, np.float64),
                             np.asarray(U2, np.float64))
    UA, UB, G = _build_u(U3c, U2c, np.asarray(U1, np.float32))
    sm2 = _build_sels()
    w_comb = np.einsum('lde,lec->ldc', np.asarray(w_fc2, np.float32),
                       np.asarray(w_lin, np.float32))
    w1t = np.concatenate([np.asarray(w_fc1, np.float32)[l].T
                          for l in range(3)], axis=1)
    wct = np.concatenate([w_comb[l].T for l in range(3)], axis=1)
    w3g = np.asarray(W3, np.float32)[a_n]          # [N, 16, 64]
    w2g = np.asarray(W2, np.float32)[a_n]
    w1g = np.asarray(W1, np.float32)[a_n]
    F = nloc * CD

    def put(buf, nm, arr, r0=0):
        o = PK_OFF[nm]
        arr = np.asarray(arr, np.float32).astype(np.float16)
        buf[r0:r0 + arr.shape[0], o:o + arr.shape[1]] = arr

    b12 = np.stack([np.asarray(b_fc1, np.float32),
                    np.asarray(b_fc2, np.float32)], axis=1).astype(np.float32)
    in_maps = []
    for core in range(ncores):
        s = slice(core * nloc, (core + 1) * nloc)
        parts = []
        for l in range(3):
            seg = irreps_x[s, l * l:(l + 1) * (l + 1), :]   # [nloc, w, 64]
            parts.append(seg.transpose(2, 1, 0).reshape(CD, -1))
        xtc = np.concatenate(parts, axis=1)                 # [64, 9*nloc]
        pk = np.zeros((128, PK_BASE + MD * nloc), np.float16)
        put(pk, "ua", UA)
        put(pk, "ub", UB)
        put(pk, "glo", G[0:128])
        put(pk, "ghi", G[128:MOUT])
        put(pk, "w1t", w1t)
        put(pk, "wct", wct)
        put(pk, "sm2", sm2)
        pk[:CD, PK_BASE:PK_BASE + MD * nloc] = xtc.astype(np.float16)
        wg3 = w3g[s].transpose(1, 2, 0).reshape(P3D, F)     # [16, F] f=(c,n)
        wg2 = w2g[s].transpose(1, 2, 0).reshape(P2D, F)
        wg1 = w1g[s].transpose(1, 2, 0).reshape(P1D, F)
        wrep = np.concatenate([np.tile(wg3, (MD, 1)), np.tile(wg2, (MD, 1)),
                               np.tile(wg1, (MD, 1))], axis=0)  # [198, F]
        in_maps.append({
            "pk": pk,
            "b12": b12,
            "wrep": wrep.astype(np.float16),
        })
    return in_maps


def unpack_out(o, nloc=NLOC):
    # o: [64, 9*nloc] cols (o, n) o-major -> [nloc, 9, 64]
    return np.ascontiguousarray(
        o.reshape(CD, MD, nloc).transpose(2, 1, 0)).astype(np.float32)


# ---------------------------------------------------------------- entry
def kernel(**inputs):
    from concourse import bass_utils
    in_maps = make_in_maps(**inputs)
    nc = _get_program(NLOC)
    res = bass_utils.run_bass_kernel_spmd(nc, in_maps,
                                          core_ids=list(range(NCORES)))
    outs = [unpack_out(res.results[c]["out"]) for c in range(NCORES)]
    return np.concatenate(outs, axis=0).astype(np.float32)
